# revision 3
# baseline (speedup 1.0000x reference)
"""Trainium2 Bass kernel for DigitConvolutionalModel forward pass.

Model: x[B,784] -> 3x3 valid conv (28x28 -> 26x26) -> flatten[676]
       -> Linear(676->200) + ReLU -> Linear(200->10).

Key algebraic optimization: the conv is linear and feeds straight into the
first Linear, so both fold into a single effective weight
W_eff[200,784] = w0 compose conv  (computed once on host, ~1.2 MFLOP).
The device then runs two dense GEMMs per batch shard:
    h = relu(x @ W_eff.T + b0);  out = h @ w1.T + b1

Sharding: pure data parallel over the batch dim across 8 NeuronCores
(4096 rows each); weights replicated; no collectives (forward only).

On-device layout is feature-major ("transposed") so the contraction dim
always lives on SBUF partitions: xT[784,n] -> hT[200,n] -> outT[10,n].

DMA schedule (the startup critical path):
 - every x segment is split into two half-width transfers that ride the
   two HWDGE rings (SP + ACT) in parallel -> per-segment flight is
   halved and both rings advance in segment order (no round-robin
   stealing from the segment the PE needs next; rings are kept strictly
   serial via explicit DMA-to-DMA deps).
 - the whole shard stays resident in SBUF (xin bufs = n segments) so no
   x DMA is ever gated on compute progress.
 - the first k-tile of w0's m0 half rides SP ahead of all x (28 KB) so
   the very first real matmul is gated only on a 100 KB x half-segment.
 - the rest of the weights ride the SWDGE ring (bulk rate ~230 GB/s),
   as do the mid-stream output stores; layer-2 bias-add runs on the
   vector engine so the scalar engine's queue is pure x DMA issue.
Micro-warmup matmuls (64 rows) on zeroed scratch bridge the DMA flight
and trip the PE's HAM clock gate with fine granularity. Compute dtype
bf16 (1 cyc/row matmuls); PSUM accumulates f32; bias+ReLU fused on the
vector engine.
"""

import os
import sys
import types
import numpy as np

for _p in ("/opt/trn_rl_repo", "/root/.axon_site"):
    if os.path.isdir(_p) and _p not in sys.path:
        sys.path.insert(0, _p)

import concourse.bass as bass  # noqa: E402
import concourse.tile as tile  # noqa: E402
import concourse.mybir as mybir  # noqa: E402
from concourse import bacc  # noqa: E402
from concourse.bass_utils import run_bass_kernel_spmd  # noqa: E402

B = 32768
N_CORES = 8
SHARD = B // N_CORES          # 4096
KDIM = 784                    # 28*28 input features (conv folded in)
HID = 200
OUT = 10
# batch-column widths per pipeline segment: small at the head (first
# matmul operands land ASAP, compute bridges the remaining DMA flight)
# and tiny at the tail (short relu->fc2->store latency chain)
SEGS = [128, 192, 256, 512, 512, 512, 512, 512, 512, 384, 64]
KT = 112                      # k-tile partition size (7 * 112 = 784)
NKT = KDIM // KT              # 7 k-tiles
M_TILES = [(0, 128), (128, 72)]  # hidden 200 = 128 + 72 PSUM partition tiles
N_WARMUP = 26                 # micro warmup matmuls (64 rows each)
WARM_ROWS = 64

MM_DT = mybir.dt.bfloat16

last_exec_time_ns = None      # set when BASS_KERNEL_PROFILE=1


def _seg_halves(w):
    """Split a segment's batch columns across the two HWDGE rings."""
    if w <= 64:
        return [w]            # tail segment: single transfer
    return [w // 2, w - w // 2]


def _install_ntff_hook():
    """Register the axon NTFF profile hook if the image's antenv lacks it."""
    try:
        from antenv.axon_hooks import get_axon_ntff_profile_hook  # noqa: F401
        return
    except ImportError:
        pass
    try:
        from trn_agent_boot.trn_boot import _ntff_profile_via_ctypes
        hook = _ntff_profile_via_ctypes("/opt/axon/libaxon_pjrt.so")
    except Exception:
        hook = None
    mod = types.ModuleType("antenv.axon_hooks")
    mod.get_axon_ntff_profile_hook = lambda: hook
    mod.set_axon_ntff_profile_hook = lambda h: None
    sys.modules["antenv.axon_hooks"] = mod


def _np_mm_dtype():
    if MM_DT == mybir.dt.bfloat16:
        import ml_dtypes
        return np.dtype(ml_dtypes.bfloat16)
    return np.dtype(np.float32)


def fold_conv_into_fc(conv_w: np.ndarray, w0: np.ndarray) -> np.ndarray:
    """W_eff[200,784] such that x @ W_eff.T == fc1(flatten(conv(x)))."""
    w0v = w0.reshape(HID, 26, 26).astype(np.float64)
    w_img = np.zeros((HID, 28, 28), dtype=np.float64)
    for ki in range(3):
        for kj in range(3):
            w_img[:, ki:ki + 26, kj:kj + 26] += w0v * np.float64(conv_w[ki, kj])
    return w_img.reshape(HID, KDIM).astype(np.float32)


def pack_shard(xs: np.ndarray, mm_np):
    """Pack one x shard [4096, 784] into per-half-segment SBUF images.

    Segment g half h covers batch cols [c0+off, c0+off+wh):
      xg[p, a, n] = x[c0 + off + n, a*KT + p]
    Every partition line is one contiguous (a, n) run.
    """
    xsv = xs.reshape(SHARD, NKT, KT)
    arrays = {}
    c0 = 0
    for g, w in enumerate(SEGS):
        off = 0
        for h, wh in enumerate(_seg_halves(w)):
            blk = xsv[c0 + off:c0 + off + wh]           # [n, a, p]
            arrays[f"xg{g}_{h}"] = np.ascontiguousarray(
                blk.transpose(2, 1, 0).astype(mm_np))   # [p, a, n]
            off += wh
        c0 += w
    return arrays


def pack_weights(w_eff: np.ndarray, w1: np.ndarray, b0, b1, mm_np):
    """Pack weights/biases into SBUF images (w0 split for startup)."""
    # w0sb[p, a, m] = W_eff[m, a*KT + p]
    w0sb = np.ascontiguousarray(
        w_eff.reshape(HID, NKT, KT).transpose(2, 1, 0).astype(mm_np))
    w0a_k0 = np.ascontiguousarray(w0sb[:, 0:1, 0:128])   # first k-tile, m0
    w0a_rest = np.ascontiguousarray(w0sb[:, 1:, 0:128])  # k-tiles 1.. , m0
    w0b = np.ascontiguousarray(w0sb[:, :, 128:HID])      # m1 half
    # w1sb[p, 0:10] = w1[:, p].T ; w1sb[0:72, 10:20] = w1[:, 128+p].T
    w1sb = np.zeros((128, 2 * OUT), dtype=mm_np)
    w1sb[:, :OUT] = w1[:, 0:128].T.astype(mm_np)
    w1sb[:HID - 128, OUT:] = w1[:, 128:HID].T.astype(mm_np)
    # bias[p, 0] = b0[p]; bias[0:72, 1] = b0[128:200]; bias[0:10, 2] = b1
    biases = np.zeros((128, 3), dtype=np.float32)
    biases[:, 0] = b0[0:128]
    biases[:HID - 128, 1] = b0[128:HID]
    biases[:OUT, 2] = b1
    return w0a_k0, w0a_rest, w0b, w1sb, biases


def build_program():
    nc = bacc.Bacc("TRN2", target_bir_lowering=False, debug=False)
    f32 = mybir.dt.float32
    add = mybir.AluOpType.add
    amax = mybir.AluOpType.max

    xg_d = {}
    for g, w in enumerate(SEGS):
        for h, wh in enumerate(_seg_halves(w)):
            xg_d[(g, h)] = nc.declare_dram_parameter(
                f"xg{g}_{h}", [KT, NKT, wh], MM_DT, isOutput=False)
    w0ak0_d = nc.declare_dram_parameter("w0a_k0", [KT, 1, 128], MM_DT, isOutput=False)
    w0ar_d = nc.declare_dram_parameter("w0a_rest", [KT, NKT - 1, 128], MM_DT, isOutput=False)
    w0b_d = nc.declare_dram_parameter("w0b", [KT, NKT, HID - 128], MM_DT, isOutput=False)
    w1_d = nc.declare_dram_parameter("w1sb", [128, 2 * OUT], MM_DT, isOutput=False)
    bia_d = nc.declare_dram_parameter("biases", [128, 3], f32, isOutput=False)
    out_d = nc.declare_dram_parameter("out", [OUT, SHARD], f32, isOutput=True)

    with tile.TileContext(nc) as tc:
        with (
            tc.tile_pool(name="weights", bufs=1) as wpool,
            # whole shard resident: no x DMA is ever gated on compute
            tc.tile_pool(name="xin", bufs=len(SEGS)) as xpool,
            tc.tile_pool(name="hbuf", bufs=2) as hpool,
            tc.tile_pool(name="obuf", bufs=4) as opool,
            tc.tile_pool(name="psum", bufs=2, space=bass.MemorySpace.PSUM) as pp,
            tc.tile_pool(name="opsum", bufs=2, space=bass.MemorySpace.PSUM) as op,
        ):
            chains = {id(nc.sync): [], id(nc.scalar): [], id(nc.gpsimd): []}

            def chained_dma(eng, dst_ap, src_ap):
                dma = eng.dma_start(dst_ap, src_ap)
                chain = chains[id(eng)]
                if chain:
                    # strict serial per ring: the SDMA engines round-robin
                    # across queued transfers, which would let segment g+1
                    # steal bandwidth from the segment the PE needs next
                    tile.add_dep_helper(
                        dma.ins, chain[-1].ins, sync=True,
                        reason="serialize ring")
                chain.append(dma)
                return dma

            # w0 m0 half: first k-tile rides SP ahead of all x (28 KB) so
            # the first matmul is gated only on a 100 KB x half-segment;
            # the rest of the weights ride SWDGE (bulk ~230 GB/s)
            w0t = [wpool.tile([KT, NKT, dm], MM_DT, name=f"w0_{mi}")
                   for mi, (m0, dm) in enumerate(M_TILES)]
            chained_dma(nc.sync, w0t[0][:, 0:1, :], w0ak0_d[:])
            chained_dma(nc.gpsimd, w0t[0][:, 1:, :], w0ar_d[:])
            chained_dma(nc.gpsimd, w0t[1][:], w0b_d[:])
            bia = wpool.tile([128, 3], f32)
            chained_dma(nc.gpsimd, bia[:], bia_d[:])
            w1 = wpool.tile([128, 2 * OUT], MM_DT)
            chained_dma(nc.gpsimd, w1[:], w1_d[:])

            # x half-segments: halves of each segment ride SP + ACT in
            # parallel, so both rings advance in segment order
            xg_t = []
            for g, w in enumerate(SEGS):
                xg = xpool.tile([KT, NKT, w], MM_DT, tag="xg", name=f"xg_{g}")
                xg_t.append(xg)
                off = 0
                halves = _seg_halves(w)
                for h, wh in enumerate(halves):
                    eng = nc.sync if h == 0 else nc.scalar
                    chained_dma(eng, xg[:, :, off:off + wh], xg_d[(g, h)][:])
                    off += wh

            # PE pre-warm on zeroed scratch while the first DMAs fly:
            # many small matmuls so real work starts the moment data lands
            warm_x = wpool.tile([KT, 128], MM_DT)
            nc.gpsimd.memset(warm_x[:], 0.0)
            warm_ps = op.tile([128, WARM_ROWS], f32, tag="warm", bufs=1)
            for _ in range(N_WARMUP):
                nc.tensor.matmul(
                    warm_ps[:], warm_x[:, 0:128], warm_x[:, 0:WARM_ROWS],
                    start=True, stop=True)

            def emit_layer2(g, w, c0, h_tiles):
                # layer 2: outT[10, seg], 2 accumulating matmuls
                o_ps = op.tile([OUT, w], f32, tag="ops", name=f"ops_{g}")
                nc.tensor.matmul(
                    o_ps[:], w1[0:128, 0:OUT], h_tiles[0][:],
                    start=True, stop=False)
                nc.tensor.matmul(
                    o_ps[:], w1[0:HID - 128, OUT:2 * OUT], h_tiles[1][:],
                    start=False, stop=True)
                o_sb = opool.tile([OUT, w], f32, tag="osb", name=f"osb_{g}")
                # bias-add on the vector engine (scalar's queue must stay
                # pure x-DMA issue so a stalled DMA never delays layer 2)
                nc.vector.tensor_scalar_add(o_sb[:], o_ps[:], bia[0:OUT, 2:3])
                # mid-stream output stores ride SWDGE (idle, off the
                # critical path); the last store rides SP which is idle
                # by then (low latency into the postamble fence)
                if g == len(SEGS) - 1:
                    nc.sync.dma_start(out_d[:, c0:c0 + w], o_sb[:])
                else:
                    eng = nc.gpsimd
                    dma = eng.dma_start(out_d[:, c0:c0 + w], o_sb[:])
                    chains[id(eng)].append(dma)

            c0 = 0
            pending = None   # layer 2 runs one segment behind layer 1,
            # so the PE never waits on the DVE relu at a seg boundary
            for g, w in enumerate(SEGS):
                xg = xg_t[g]
                # layer 1: hT[m0:m0+dm, seg], 7 accumulating matmuls
                h_tiles = []
                for mi, (m0, dm) in enumerate(M_TILES):
                    h_ps = pp.tile([dm, w], f32, tag=f"hps{mi}",
                                   name=f"hps_{g}_{mi}")
                    for a in range(NKT):
                        nc.tensor.matmul(
                            h_ps[:],
                            w0t[mi][:, a, :],
                            xg[:, a, :],
                            start=(a == 0),
                            stop=(a == NKT - 1),
                        )
                    h_sb = hpool.tile([dm, w], MM_DT, tag=f"h{mi}",
                                      name=f"h_{g}_{mi}")
                    # fused bias + relu on the vector engine
                    nc.vector.tensor_scalar(
                        h_sb[:], h_ps[:], bia[0:dm, mi:mi + 1], 0.0,
                        add, amax)
                    h_tiles.append(h_sb)

                if pending is not None:
                    emit_layer2(*pending)
                pending = (g, w, c0, h_tiles)
                c0 += w

            emit_layer2(*pending)

    nc.compile()
    return nc


_program_cache = {}


def _get_program():
    key = (MM_DT, tuple(SEGS), N_WARMUP)
    if key not in _program_cache:
        _program_cache[key] = build_program()
    return _program_cache[key]


def kernel(**inputs: np.ndarray) -> np.ndarray:
    x = np.asarray(inputs["x"], dtype=np.float32)
    conv_w = np.asarray(inputs["conv_w"], dtype=np.float32)
    w0 = np.asarray(inputs["w0"], dtype=np.float32)
    b0 = np.asarray(inputs["b0"], dtype=np.float32)
    w1 = np.asarray(inputs["w1"], dtype=np.float32)
    b1 = np.asarray(inputs["b1"], dtype=np.float32)

    mm_np = _np_mm_dtype()
    w_eff = fold_conv_into_fc(conv_w, w0)
    w0a_k0, w0a_rest, w0b, w1sb, biases = pack_weights(w_eff, w1, b0, b1, mm_np)

    in_maps = []
    for i in range(N_CORES):
        m = pack_shard(x[i * SHARD:(i + 1) * SHARD], mm_np)
        m.update({"w0a_k0": w0a_k0, "w0a_rest": w0a_rest, "w0b": w0b,
                  "w1sb": w1sb, "biases": biases})
        in_maps.append(m)

    nc = _get_program()

    profile = os.environ.get("BASS_KERNEL_PROFILE", "0") == "1"
    kwargs = {}
    if profile:
        _install_ntff_hook()
        kwargs = dict(trace=True, tmpdir=os.environ.get("BASS_KERNEL_TRACE_DIR"))
    try:
        res = run_bass_kernel_spmd(
            nc, in_maps, core_ids=list(range(N_CORES)), **kwargs)
    except Exception:
        # a previous process can leave a NeuronCore momentarily
        # unrecoverable (NRT_EXEC_UNIT_UNRECOVERABLE); one retry suffices
        import time
        time.sleep(5)
        res = run_bass_kernel_spmd(
            nc, in_maps, core_ids=list(range(N_CORES)), **kwargs)

    global last_exec_time_ns
    last_exec_time_ns = res.exec_time_ns

    out = np.empty((B, OUT), dtype=np.float32)
    for i in range(N_CORES):
        out[i * SHARD:(i + 1) * SHARD] = res.results[i]["out"].T
    return out


# revision 4
# speedup vs baseline: 1.1457x; 1.1457x over previous
"""Trainium2 Bass kernel for DigitConvolutionalModel forward pass.

Model: x[B,784] -> 3x3 valid conv (28x28 -> 26x26) -> flatten[676]
       -> Linear(676->200) + ReLU -> Linear(200->10).

Key algebraic optimization: the conv is linear and feeds straight into the
first Linear, so both fold into a single effective weight
W_eff[200,784] = w0 compose conv  (computed once on host, ~1.2 MFLOP).
The device then runs two dense GEMMs per batch shard:
    h = relu(x @ W_eff.T + b0);  out = h @ w1.T + b1

Sharding: pure data parallel over the batch dim across 8 NeuronCores
(4096 rows each); weights replicated; no collectives (forward only).

On-device layout is feature-major ("transposed") so the contraction dim
always lives on SBUF partitions: xT[784,n] -> hT[200,n] -> outT[10,n].

DMA schedule (the startup critical path):
 - every x segment is stored as [KT, 2, NKT, w/2]: each half is one
   fully-contiguous 2D transfer, and the two halves ride the two HWDGE
   rings (SP + ACT) in parallel -> per-segment flight is halved. A
   single matmul still consumes the whole segment through a 3-dim
   moving AP xg[:, :, a, :] (the halves concatenate in stream order,
   which is exactly batch order).
 - per-ring DMA depth is capped at 2 with explicit DMA-to-DMA deps
   (the SDMA engines round-robin across queued transfers; uncapped
   depth delays the segment the PE needs next, depth 1 leaves the ring
   idle between transfers).
 - the whole shard stays resident in SBUF (xin bufs = n segments) so no
   x DMA is ever gated on compute progress.
 - w0's m0 half is split across the heads of both rings (k-tiles 0-2 on
   SP, 3-6 on ACT) so the first real matmul is gated only on a ~100 KB
   flight; w0's m1 half, biases and w1 ride SWDGE in need-order, as do
   the mid-stream output stores; layer-2 bias-add runs on the vector
   engine so the scalar engine's queue is pure x DMA issue.
Micro-warmup matmuls (64 rows) on zeroed scratch bridge the DMA flight
and trip the PE's HAM clock gate with fine granularity. Compute dtype
bf16 (1 cyc/row matmuls); PSUM accumulates f32; bias+ReLU fused on the
vector engine.
"""

import os
import sys
import types
import numpy as np

for _p in ("/opt/trn_rl_repo", "/root/.axon_site"):
    if os.path.isdir(_p) and _p not in sys.path:
        sys.path.insert(0, _p)

import concourse.bass as bass  # noqa: E402
import concourse.tile as tile  # noqa: E402
import concourse.mybir as mybir  # noqa: E402
from concourse import bacc  # noqa: E402
from concourse.bass_utils import run_bass_kernel_spmd  # noqa: E402

B = 32768
N_CORES = 8
SHARD = B // N_CORES          # 4096
KDIM = 784                    # 28*28 input features (conv folded in)
HID = 200
OUT = 10
# batch-column widths per pipeline segment: small at the head (first
# matmul operands land ASAP, compute bridges the remaining DMA flight)
# and tiny at the tail (short relu->fc2->store latency chain)
SEGS = [128, 192, 256, 512, 512, 512, 512, 512, 512, 384, 64]
KT = 112                      # k-tile partition size (7 * 112 = 784)
NKT = KDIM // KT              # 7 k-tiles
M_TILES = [(0, 128), (128, 72)]  # hidden 200 = 128 + 72 PSUM partition tiles
W0_KSPLIT = 3                 # w0 m0 k-tiles 0:3 ride SP, 3:7 ride ACT
N_WARMUP = 22                 # micro warmup matmuls (64 rows each)
WARM_ROWS = 64

MM_DT = mybir.dt.bfloat16

last_exec_time_ns = None      # set when BASS_KERNEL_PROFILE=1


def _install_ntff_hook():
    """Register the axon NTFF profile hook if the image's antenv lacks it."""
    try:
        from antenv.axon_hooks import get_axon_ntff_profile_hook  # noqa: F401
        return
    except ImportError:
        pass
    try:
        from trn_agent_boot.trn_boot import _ntff_profile_via_ctypes
        hook = _ntff_profile_via_ctypes("/opt/axon/libaxon_pjrt.so")
    except Exception:
        hook = None
    mod = types.ModuleType("antenv.axon_hooks")
    mod.get_axon_ntff_profile_hook = lambda: hook
    mod.set_axon_ntff_profile_hook = lambda h: None
    sys.modules["antenv.axon_hooks"] = mod


def _np_mm_dtype():
    if MM_DT == mybir.dt.bfloat16:
        import ml_dtypes
        return np.dtype(ml_dtypes.bfloat16)
    return np.dtype(np.float32)


def fold_conv_into_fc(conv_w: np.ndarray, w0: np.ndarray) -> np.ndarray:
    """W_eff[200,784] such that x @ W_eff.T == fc1(flatten(conv(x)))."""
    w0v = w0.reshape(HID, 26, 26).astype(np.float64)
    w_img = np.zeros((HID, 28, 28), dtype=np.float64)
    for ki in range(3):
        for kj in range(3):
            w_img[:, ki:ki + 26, kj:kj + 26] += w0v * np.float64(conv_w[ki, kj])
    return w_img.reshape(HID, KDIM).astype(np.float32)


def pack_shard(xs: np.ndarray, mm_np):
    """Pack one x shard [4096, 784] into per-half-segment SBUF images.

    Segment g half h covers batch cols [c0 + h*w/2, c0 + (h+1)*w/2):
      xg{g}_{h}[p, a, n] = x[c0 + h*w/2 + n, a*KT + p]
    Every partition line is one contiguous (a, n) run.
    """
    xsv = xs.reshape(SHARD, NKT, KT)
    arrays = {}
    c0 = 0
    for g, w in enumerate(SEGS):
        wh = w // 2
        for h in range(2):
            blk = xsv[c0 + h * wh:c0 + (h + 1) * wh]    # [n, a, p]
            arrays[f"xg{g}_{h}"] = np.ascontiguousarray(
                blk.transpose(2, 1, 0).astype(mm_np))   # [p, a, n]
        c0 += w
    return arrays


def pack_weights(w_eff: np.ndarray, w1: np.ndarray, b0, b1, mm_np):
    """Pack weights/biases into SBUF images (w0 split for startup)."""
    # w0sb[p, a, m] = W_eff[m, a*KT + p]
    w0sb = np.ascontiguousarray(
        w_eff.reshape(HID, NKT, KT).transpose(2, 1, 0).astype(mm_np))
    w0a_lo = np.ascontiguousarray(w0sb[:, :W0_KSPLIT, 0:128])
    w0a_hi = np.ascontiguousarray(w0sb[:, W0_KSPLIT:, 0:128])
    w0b = np.ascontiguousarray(w0sb[:, :, 128:HID])      # m1 half
    # w1sb[p, 0:10] = w1[:, p].T ; w1sb[0:72, 10:20] = w1[:, 128+p].T
    w1sb = np.zeros((128, 2 * OUT), dtype=mm_np)
    w1sb[:, :OUT] = w1[:, 0:128].T.astype(mm_np)
    w1sb[:HID - 128, OUT:] = w1[:, 128:HID].T.astype(mm_np)
    # bias[p, 0] = b0[p]; bias[0:72, 1] = b0[128:200]; bias[0:10, 2] = b1
    biases = np.zeros((128, 3), dtype=np.float32)
    biases[:, 0] = b0[0:128]
    biases[:HID - 128, 1] = b0[128:HID]
    biases[:OUT, 2] = b1
    return w0a_lo, w0a_hi, w0b, w1sb, biases


def build_program():
    nc = bacc.Bacc("TRN2", target_bir_lowering=False, debug=False)
    f32 = mybir.dt.float32
    add = mybir.AluOpType.add
    amax = mybir.AluOpType.max

    xg_d = {}
    for g, w in enumerate(SEGS):
        for h in range(2):
            xg_d[(g, h)] = nc.declare_dram_parameter(
                f"xg{g}_{h}", [KT, NKT, w // 2], MM_DT, isOutput=False)
    w0alo_d = nc.declare_dram_parameter(
        "w0a_lo", [KT, W0_KSPLIT, 128], MM_DT, isOutput=False)
    w0ahi_d = nc.declare_dram_parameter(
        "w0a_hi", [KT, NKT - W0_KSPLIT, 128], MM_DT, isOutput=False)
    w0b_d = nc.declare_dram_parameter("w0b", [KT, NKT, HID - 128], MM_DT, isOutput=False)
    w1_d = nc.declare_dram_parameter("w1sb", [128, 2 * OUT], MM_DT, isOutput=False)
    bia_d = nc.declare_dram_parameter("biases", [128, 3], f32, isOutput=False)
    out_d = nc.declare_dram_parameter("out", [OUT, SHARD], f32, isOutput=True)

    with tile.TileContext(nc) as tc:
        with (
            tc.tile_pool(name="weights", bufs=1) as wpool,
            # whole shard resident: no x DMA is ever gated on compute
            tc.tile_pool(name="xin", bufs=len(SEGS)) as xpool,
            tc.tile_pool(name="hbuf", bufs=2) as hpool,
            tc.tile_pool(name="obuf", bufs=4) as opool,
            tc.tile_pool(name="psum", bufs=2, space=bass.MemorySpace.PSUM) as pp,
            tc.tile_pool(name="opsum", bufs=2, space=bass.MemorySpace.PSUM) as op,
        ):
            chains = {id(nc.sync): [], id(nc.scalar): [], id(nc.gpsimd): []}

            def chained_dma(eng, dst_ap, src_ap):
                dma = eng.dma_start(dst_ap, src_ap)
                chain = chains[id(eng)]
                if len(chain) >= 2:
                    # cap per-ring depth at 2: keeps the ring busy across
                    # completion->issue gaps without letting later
                    # transfers round-robin-steal from the one the PE
                    # needs next
                    tile.add_dep_helper(
                        dma.ins, chain[-2].ins, sync=True,
                        reason="cap ring depth at 2")
                chain.append(dma)
                return dma

            # w0 m0 half split across the heads of both HWDGE rings so the
            # first matmuls' weights land with the first x half-segments;
            # m1 half + small operands ride SWDGE in need-order
            w0t = [wpool.tile([KT, NKT, dm], MM_DT, name=f"w0_{mi}")
                   for mi, (m0, dm) in enumerate(M_TILES)]
            chained_dma(nc.sync, w0t[0][:, :W0_KSPLIT, :], w0alo_d[:])
            chained_dma(nc.scalar, w0t[0][:, W0_KSPLIT:, :], w0ahi_d[:])
            chained_dma(nc.gpsimd, w0t[1][:], w0b_d[:])
            bia = wpool.tile([128, 3], f32)
            chained_dma(nc.gpsimd, bia[:], bia_d[:])
            w1 = wpool.tile([128, 2 * OUT], MM_DT)
            chained_dma(nc.gpsimd, w1[:], w1_d[:])

            # x segments: halves ride SP + ACT in parallel, stored as
            # [KT, 2, NKT, w/2] so each half is one contiguous transfer
            xg_t = []
            for g, w in enumerate(SEGS):
                xg = xpool.tile([KT, 2, NKT, w // 2], MM_DT, tag="xg",
                                name=f"xg_{g}")
                xg_t.append(xg)
                for h, eng in enumerate((nc.sync, nc.scalar)):
                    chained_dma(eng, xg[:, h, :, :], xg_d[(g, h)][:])

            # PE pre-warm on zeroed scratch while the first DMAs fly:
            # many small matmuls so real work starts the moment data lands
            warm_x = wpool.tile([KT, 128], MM_DT)
            nc.gpsimd.memset(warm_x[:], 0.0)
            warm_ps = op.tile([128, WARM_ROWS], f32, tag="warm", bufs=1)
            for _ in range(N_WARMUP):
                nc.tensor.matmul(
                    warm_ps[:], warm_x[:, 0:128], warm_x[:, 0:WARM_ROWS],
                    start=True, stop=True)

            def emit_layer2(g, w, c0, h_tiles):
                # layer 2: outT[10, seg], 2 accumulating matmuls
                o_ps = op.tile([OUT, w], f32, tag="ops", name=f"ops_{g}")
                nc.tensor.matmul(
                    o_ps[:], w1[0:128, 0:OUT], h_tiles[0][:],
                    start=True, stop=False)
                nc.tensor.matmul(
                    o_ps[:], w1[0:HID - 128, OUT:2 * OUT], h_tiles[1][:],
                    start=False, stop=True)
                o_sb = opool.tile([OUT, w], f32, tag="osb", name=f"osb_{g}")
                # bias-add on the vector engine (scalar's queue must stay
                # pure x-DMA issue so a stalled DMA never delays layer 2)
                nc.vector.tensor_scalar_add(o_sb[:], o_ps[:], bia[0:OUT, 2:3])
                # mid-stream output stores ride SWDGE (idle, off the
                # critical path); the last store rides SP which is idle
                # by then (low latency into the postamble fence)
                if g == len(SEGS) - 1:
                    nc.sync.dma_start(out_d[:, c0:c0 + w], o_sb[:])
                else:
                    dma = nc.gpsimd.dma_start(out_d[:, c0:c0 + w], o_sb[:])
                    chains[id(nc.gpsimd)].append(dma)

            c0 = 0
            pending = None   # layer 2 runs one segment behind layer 1,
            # so the PE never waits on the DVE relu at a seg boundary
            for g, w in enumerate(SEGS):
                xg = xg_t[g]
                # layer 1: hT[m0:m0+dm, seg], 7 accumulating matmuls
                h_tiles = []
                for mi, (m0, dm) in enumerate(M_TILES):
                    h_ps = pp.tile([dm, w], f32, tag=f"hps{mi}",
                                   name=f"hps_{g}_{mi}")
                    for a in range(NKT):
                        nc.tensor.matmul(
                            h_ps[:],
                            w0t[mi][:, a, :],
                            xg[:, :, a, :],
                            start=(a == 0),
                            stop=(a == NKT - 1),
                        )
                    h_sb = hpool.tile([dm, w], MM_DT, tag=f"h{mi}",
                                      name=f"h_{g}_{mi}")
                    # fused bias + relu on the vector engine
                    nc.vector.tensor_scalar(
                        h_sb[:], h_ps[:], bia[0:dm, mi:mi + 1], 0.0,
                        add, amax)
                    h_tiles.append(h_sb)

                if pending is not None:
                    emit_layer2(*pending)
                pending = (g, w, c0, h_tiles)
                c0 += w

            emit_layer2(*pending)

    nc.compile()
    return nc


_program_cache = {}


def _get_program():
    key = (MM_DT, tuple(SEGS), N_WARMUP)
    if key not in _program_cache:
        _program_cache[key] = build_program()
    return _program_cache[key]


def kernel(**inputs: np.ndarray) -> np.ndarray:
    x = np.asarray(inputs["x"], dtype=np.float32)
    conv_w = np.asarray(inputs["conv_w"], dtype=np.float32)
    w0 = np.asarray(inputs["w0"], dtype=np.float32)
    b0 = np.asarray(inputs["b0"], dtype=np.float32)
    w1 = np.asarray(inputs["w1"], dtype=np.float32)
    b1 = np.asarray(inputs["b1"], dtype=np.float32)

    mm_np = _np_mm_dtype()
    w_eff = fold_conv_into_fc(conv_w, w0)
    w0a_lo, w0a_hi, w0b, w1sb, biases = pack_weights(w_eff, w1, b0, b1, mm_np)

    in_maps = []
    for i in range(N_CORES):
        m = pack_shard(x[i * SHARD:(i + 1) * SHARD], mm_np)
        m.update({"w0a_lo": w0a_lo, "w0a_hi": w0a_hi, "w0b": w0b,
                  "w1sb": w1sb, "biases": biases})
        in_maps.append(m)

    nc = _get_program()

    profile = os.environ.get("BASS_KERNEL_PROFILE", "0") == "1"
    kwargs = {}
    if profile:
        _install_ntff_hook()
        kwargs = dict(trace=True, tmpdir=os.environ.get("BASS_KERNEL_TRACE_DIR"))
    try:
        res = run_bass_kernel_spmd(
            nc, in_maps, core_ids=list(range(N_CORES)), **kwargs)
    except Exception:
        # a previous process can leave a NeuronCore momentarily
        # unrecoverable (NRT_EXEC_UNIT_UNRECOVERABLE); one retry suffices
        import time
        time.sleep(5)
        res = run_bass_kernel_spmd(
            nc, in_maps, core_ids=list(range(N_CORES)), **kwargs)

    global last_exec_time_ns
    last_exec_time_ns = res.exec_time_ns

    out = np.empty((B, OUT), dtype=np.float32)
    for i in range(N_CORES):
        out[i * SHARD:(i + 1) * SHARD] = res.results[i]["out"].T
    return out
